# revision 35
# baseline (speedup 1.0000x reference)
"""Trainium2 Bass kernel for nn_BinarizedConv2d (3-bit-packed weight stream).

Math: activation[d, o] = sum_k weight_noise[d, o, k] * x[d, k]
      out[d, o]        = activation[d, o] > bias_noise[d, o]
with D=128 directions, O=256 out channels, K=2304 reduction length.
Sharding: D split across 8 NeuronCores (16 directions per core), no
collectives.

Weights and x are 0/1 bits, so THREE adjacent k-bits are packed host-side
into one fp8 byte as single-bit fields of the e4m3 ENCODING:
    enc = w0*0x40 | w1*0x20 | w2*0x08
Because each field is a single bit, (enc & mask) is always a valid fp8
float with an exact per-bit value:
    enc & 0x40 = 2.0   * w0     (exponent bit)
    enc & 0x20 = 0.125 * w1     (exponent bit)
    enc & 0x08 = 2^-6  * w2     (mantissa msb)
so three uint16-bitcast AND ops (DVE 4x perf mode) reconstruct three exact
operand streams from a 3.15 MB/core HBM stream (3 bits/byte; the kernel is
HBM-bound). The matvec is three accumulating matmul streams per direction
with host-prescaled x coefficients 0.5*x0 / 8*x1 / 64*x2 (exact fp8), so
every partial product is 0 or 1 and fp32 PSUM accumulation is exact.

The threshold is folded into PSUM by one tiny fp16 matmul per quad
(stationary selneg[j, m] = -1 iff m//32 == j, moving operand the per-quad
row of kf = floor(bias), integers ~576, exact in fp16), run FIRST with
start=True - floor(bias) is an integer so all partials stay exact. For
integer activations,  act > bias <=> act - floor(bias) > 0.5,  so the
epilogue is a single-src (psum is_gt 0.5) -> uint8 on DVE.

Scheduling (from trace analysis of the fp8/b=2 versions): ~0.7us issue
cost per dma_start and ~8 HWDGE completion semaphores; big chunks sustain
~430 GB/s where many small ones starve (~350); each chunk's completion
semaphore fires 2-4us after its bytes land (HBM receipt round-trip), so
the last chunk is kept small and the last quad's matmuls are tile-
interleaved; ~30 dummy matmuls into an unused PSUM window pre-warm the PE
clock gate (HAM lifts 1.2->2.4 GHz after ~3.4us of sustained activity);
bulk results fly out early and only 1KB rides the final DMA receipt.
"""

import numpy as np
import ml_dtypes

D = 128          # directions (ES population)
O = 256          # out channels
K = 2304         # flattened reduction length
NT = 6           # packed k-tiles of 128 (K/3 = 768 triples)
P = 128          # partitions
NCORES = 8
DPC = D // NCORES  # directions per core
NQ = DPC // 4      # quads per core
NS = 3             # bit-streams per packed byte

FP8 = ml_dtypes.float8_e4m3
MASKS = (0x4040, 0x2020, 0x0808)
SCALES = (0.5, 8.0, 64.0)   # coefficient prescale per stream (host side)

_nc_cache = {}

# weight chunk schedule: (quad, tile0, tile1) in consume order
CHUNKS = [
    (0, 0, 2), (0, 2, 4), (0, 4, 6),
    (1, 0, 2), (1, 2, 4), (1, 4, 6),
    (2, 0, 2), (2, 2, 4), (2, 4, 6),
    (3, 0, 2), (3, 2, 4), (3, 4, 6),
]
RING_OF = [0, 0, 0, 0, 1, 0, 1, 0, 1, 0, 1, 0]


def _emit(tc, res_ap, wT_ap, xT_ap, hdr_ap):
    """Emit the per-core program into TileContext tc."""
    import concourse.mybir as mybir

    nc = tc.nc
    fp8 = mybir.dt.float8e4
    u16 = mybir.dt.uint16
    f16 = mybir.dt.float16
    f32 = mybir.dt.float32
    u8 = mybir.dt.uint8
    XN = DPC * NT  # 96 coefficient columns per stream

    with (
        tc.tile_pool(name="w", bufs=1) as wp,
        tc.tile_pool(name="small", bufs=1) as sp,
        tc.tile_pool(name="act", bufs=1) as ap_pool,
        tc.tile_pool(name="ps", bufs=1, space="PSUM") as pp,
    ):
        # prescaled x coefficient streams, first on the SP ring:
        # xc[:, s*XN + d*NT + t] = SCALES[s] * x[d0+d, 3*(t*128+p) + s]
        xc = sp.tile([P, NS * XN], fp8)
        nc.sync.dma_start(out=xc[:], in_=xT_ap)
        # header on the ACT ring: kf = floor(bias) [4, NQ*O] ++ selneg [4,128]
        hdr = sp.tile([4, NQ * O + P], f16)
        nc.scalar.dma_start(out=hdr[:], in_=hdr_ap)

        ring = [nc.sync, nc.scalar]
        p_tiles = [wp.tile([P, NT * 4 * O], fp8, tag=f"p{q}", name=f"p_t{q}")
                   for q in range(NQ)]
        s_tiles = []
        for s in range(NS):
            row = []
            for q in range(NQ):
                t_ = wp.tile([P, NT * 4 * O], fp8, tag=f"s{s}q{q}",
                             name=f"s_t{s}_{q}")
                row.append(t_)
            s_tiles.append(row)
        for ci, (qi, t0, t1) in enumerate(CHUNKS):
            c0, c1 = t0 * 4 * O, t1 * 4 * O
            ring[RING_OF[ci]].dma_start(
                out=p_tiles[qi][:, c0:c1], in_=wT_ap[qi][:, c0:c1]
            )

        res_all = ap_pool.tile([P, NQ * O], u8)
        ps_all = pp.tile([P, 8 * 2 * O], f32)
        probe = sp.tile([1, 4], f32)

        # PE warm-up (HAM clock gate): ~3.5us of dummy matmuls into an
        # unused PSUM window before the first weight chunk lands.
        scratch = sp.tile([P, 2 * O], fp8)
        nc.vector.memset(scratch[:], 0.0)
        neg128 = sp.tile([P, 1], f32)
        nc.vector.memset(neg128[:], -128.0)
        for w in range(16):
            nc.tensor.matmul(
                ps_all[0:32, O : 2 * O],
                scratch[:, 0:32],
                scratch[:, 0:O],
                start=True,
                stop=True,
                tile_position=(0, 0),
                skip_group_check=True,
            )

        def derive(qi, t0, t1):
            c0, c1 = t0 * 4 * O, t1 * 4 * O
            for s in range(NS):
                nc.vector.tensor_scalar(
                    out=s_tiles[s][qi][:, c0:c1].bitcast(u16),
                    in0=p_tiles[qi][:, c0:c1].bitcast(u16),
                    scalar1=MASKS[s], scalar2=None,
                    op0=mybir.AluOpType.bitwise_and,
                )

        def mm_quad(q):
            win = slice(q * 2 * O, q * 2 * O + O)
            # -floor(bias) seeds the accumulation (integer => all partials
            # stay exact fp32 integers); keeping it FIRST removes it from
            # the kernel tail.
            nc.tensor.matmul(
                ps_all[:, win],
                hdr[0:4, NQ * O : NQ * O + P],
                hdr[0:4, q * O : (q + 1) * O],
                start=True,
                stop=False,
                skip_group_check=True,
            )
            # quads 0-2: stream-major (their derives finish mid-stream);
            # quad 3: tile-major so only the final small chunk's matmuls
            # trail the last DMA completion.
            if q == 3:
                order = [(t, s) for t in range(NT) for s in range(NS)]
            else:
                order = [(t, s) for s in range(NS) for t in range(NT)]
            for i, (t, s) in enumerate(order):
                for j in range(4):
                    d = q * 4 + j
                    nc.tensor.matmul(
                        ps_all[32 * j : 32 * (j + 1), win],
                        xc[:, s * XN + d * NT + t : s * XN + d * NT + t + 1]
                        .broadcast_to((P, 32)),
                        s_tiles[s][q][:, (t * 4 + j) * O : (t * 4 + j + 1) * O],
                        start=False,
                        stop=(i == len(order) - 1 and j == 3),
                        tile_position=(0, 32 * j),
                        skip_group_check=True,
                    )

        # Compares on the (otherwise idle) ACT engine so the DVE FIFO is a
        # pure derive chain (the derive chain is the measured critical path
        # from the first chunk semaphore to the last matmul). For integer
        # ps, act > bias <=> ps >= 1, and sigmoid(256*ps - 128) saturates
        # to exactly 1.0 / 0.0, so the uint8 cast is the exact predicate.
        # The probe triggers the one-time ~2.7us sigmoid table load early,
        # after the ACT-ring dma_starts.
        nc.scalar.activation(
            out=probe[:], in_=ps_all[0:1, O : O + 4],
            func=mybir.ActivationFunctionType.Sigmoid, scale=256.0,
            bias=neg128[0:1, :],
        )

        def compare(q):
            nc.scalar.activation(
                out=res_all[:, q * O : (q + 1) * O],
                in_=ps_all[:, q * 2 * O : q * 2 * O + O],
                func=mybir.ActivationFunctionType.Sigmoid,
                scale=256.0, bias=neg128[:],
            )

        derive(0, 0, 2)
        derive(0, 2, 4)
        derive(0, 4, 6)
        mm_quad(0)
        derive(1, 0, 2)
        derive(1, 2, 4)
        derive(1, 4, 6)
        compare(0)
        mm_quad(1)
        derive(2, 0, 4)
        derive(2, 4, 6)
        compare(1)
        mm_quad(2)
        derive(3, 0, 4)
        derive(3, 4, 6)
        compare(2)
        mm_quad(3)
        # bulk of the results flies out while quad 3 still computes; only
        # the last quad's 1KB rides the final DMA receipt
        nc.scalar.dma_start(out=res_ap[:, : 3 * O], in_=res_all[0:P:32, : 3 * O])
        compare(3)
        nc.scalar.dma_start(out=res_ap[:, 3 * O :], in_=res_all[0:P:32, 3 * O :])


def _build():
    """Build the per-core Bass program (same NEFF on all 8 cores)."""
    import concourse.bacc as bacc
    import concourse.mybir as mybir
    from concourse.tile import TileContext

    nc = bacc.Bacc("TRN2", debug=False, enable_asserts=False)

    fp8 = mybir.dt.float8e4
    f16 = mybir.dt.float16
    u8 = mybir.dt.uint8

    # wT[q, p, (t*4 + j)*O + o] = packed triple for direction d0+4q+j,
    # triple index m = t*128 + p, byte w[3m]*0x40 | w[3m+1]*0x20 | w[3m+2]*0x08
    wT = nc.dram_tensor("wT", [NQ, P, NT * 4 * O], fp8, kind="ExternalInput")
    # xT[p, s*96 + d*6 + t] = SCALES[s] * x[d0+d, 3*(t*128+p) + s]
    xT = nc.dram_tensor("xT", [P, NS * DPC * NT], fp8, kind="ExternalInput")
    # hdr[j, q*O + o] = floor(bias_noise[d0+4q+j, o]); hdr[j, NQ*O + m] =
    # -1.0 if m//32 == j else 0 (the bias-fold selector)
    hdr = nc.dram_tensor("hdr", [4, NQ * O + P], f16, kind="ExternalInput")
    # res[j, q*O + o] = out[d0+4q+j, o]
    res = nc.dram_tensor("res", [4, NQ * O], u8, kind="ExternalOutput")

    with TileContext(nc) as tc:
        _emit(tc, res.ap(), wT.ap(), xT.ap(), hdr.ap())
    nc.compile()
    return nc


def prepare_inputs(weight_noise, bias_noise, x):
    """Host-side dtype cast + bit packing + layout transform + sharding.

    All transforms are data-independent (fixed index shuffles, the exact
    0/1 bit pack into encoding fields, fixed scalar prescale of the 0/1
    x-bits, and floor() of the compare constant); the reduction/compare
    math runs on device.
    """
    w = np.asarray(weight_noise)                      # [D, O, K] 0/1 floats
    wtri = w.reshape(D, O, K // 3, 3).astype(np.uint8)
    enc = (wtri[..., 0] * 0x40 + wtri[..., 1] * 0x20 + wtri[..., 2] * 0x08
           ).astype(np.uint8).view(FP8)               # [D, O, 768]
    # [D, O, NT, P] -> [D, P, NT, O]
    pT = np.ascontiguousarray(enc.reshape(D, O, NT, P).transpose(0, 3, 2, 1))

    xb = np.asarray(x).astype(np.float32)             # [D, K] 0/1
    xs = []
    for s in range(NS):
        v = (xb[:, s::3] * SCALES[s]).astype(FP8).reshape(D, NT, P)
        xs.append(np.ascontiguousarray(v.transpose(2, 0, 1)))  # [P, D, NT]

    kf = np.floor(np.asarray(bias_noise).astype(np.float64)).astype(np.float16)
    selneg = np.zeros((4, P), dtype=np.float16)
    for j in range(4):
        selneg[j, 32 * j : 32 * (j + 1)] = -1.0

    in_maps = []
    for c in range(NCORES):
        sl = slice(c * DPC, (c + 1) * DPC)
        # [d, p, t, o] -> [q, j, p, t, o] -> [q, p, t, j, o]
        wc = (
            pT[sl]
            .reshape(NQ, 4, P, NT, O)
            .transpose(0, 2, 3, 1, 4)
            .reshape(NQ, P, NT * 4 * O)
        )
        xcs = np.concatenate(
            [xs[s][:, sl, :].reshape(P, DPC * NT) for s in range(NS)], axis=1
        )
        kc = (
            kf[sl]
            .reshape(NQ, 4, O)
            .transpose(1, 0, 2)
            .reshape(4, NQ * O)
        )
        hc = np.concatenate([kc, selneg], axis=1)
        in_maps.append(
            {
                "wT": np.ascontiguousarray(wc),
                "xT": np.ascontiguousarray(xcs),
                "hdr": np.ascontiguousarray(hc),
            }
        )
    return in_maps


def run(weight_noise, bias_noise, x, trace=False, **spmd_kwargs):
    """Run on the 8 NeuronCores; returns (bool [D, O] output, results)."""
    from concourse.bass_utils import run_bass_kernel_spmd

    in_maps = prepare_inputs(weight_noise, bias_noise, x)
    if "nc" in _nc_cache:
        nc = _nc_cache["nc"]
    else:
        nc = _nc_cache["nc"] = _build()
    r = run_bass_kernel_spmd(
        nc, in_maps, core_ids=list(range(NCORES)), trace=trace, **spmd_kwargs
    )
    out = np.concatenate(
        [
            r.results[c]["res"]
            .reshape(4, NQ, O)
            .transpose(1, 0, 2)
            .reshape(DPC, O)
            for c in range(NCORES)
        ],
        axis=0,
    )
    return out.astype(bool), r


def kernel(weight_noise, bias_noise, x):
    out, _ = run(weight_noise, bias_noise, x)
    return out


# revision 36
# speedup vs baseline: 1.0362x; 1.0362x over previous
"""Trainium2 Bass kernel for nn_BinarizedConv2d (3-bit-packed weight stream).

Math: activation[d, o] = sum_k weight_noise[d, o, k] * x[d, k]
      out[d, o]        = activation[d, o] > bias_noise[d, o]
with D=128 directions, O=256 out channels, K=2304 reduction length.
Sharding: D split across 8 NeuronCores (16 directions per core), no
collectives.

Weights and x are 0/1 bits, so THREE adjacent k-bits are packed host-side
into one fp8 byte as single-bit fields of the e4m3 ENCODING:
    enc = w0*0x40 | w1*0x20 | w2*0x08
Because each field is a single bit, (enc & mask) is always a valid fp8
float with an exact per-bit value:
    enc & 0x40 = 2.0   * w0     (exponent bit)
    enc & 0x20 = 0.125 * w1     (exponent bit)
    enc & 0x08 = 2^-6  * w2     (mantissa msb)
so three uint16-bitcast AND ops (DVE 4x perf mode) reconstruct three exact
operand streams from a 3.15 MB/core HBM stream (3 bits/byte; the kernel is
HBM-bound). The matvec is three accumulating matmul streams per direction
with host-prescaled x coefficients 0.5*x0 / 8*x1 / 64*x2 (exact fp8), so
every partial product is 0 or 1 and fp32 PSUM accumulation is exact.

The threshold is folded into PSUM by one tiny fp16 matmul per quad
(stationary selneg[j, m] = -1 iff m//32 == j, moving operand the per-quad
row of kf = floor(bias), integers ~576, exact in fp16), run FIRST with
start=True - floor(bias) is an integer so all partials stay exact. For
integer activations,  act > bias <=> act - floor(bias) > 0.5,  so the
epilogue is a single-src (psum is_gt 0.5) -> uint8 on DVE.

Scheduling (from trace analysis of the fp8/b=2 versions): ~0.7us issue
cost per dma_start and ~8 HWDGE completion semaphores; big chunks sustain
~430 GB/s where many small ones starve (~350); each chunk's completion
semaphore fires 2-4us after its bytes land (HBM receipt round-trip), so
the last chunk is kept small and the last quad's matmuls are tile-
interleaved; ~30 dummy matmuls into an unused PSUM window pre-warm the PE
clock gate (HAM lifts 1.2->2.4 GHz after ~3.4us of sustained activity);
bulk results fly out early and only 1KB rides the final DMA receipt.
"""

import numpy as np
import ml_dtypes

D = 128          # directions (ES population)
O = 256          # out channels
K = 2304         # flattened reduction length
NT = 6           # packed k-tiles of 128 (K/3 = 768 triples)
P = 128          # partitions
NCORES = 8
DPC = D // NCORES  # directions per core
NQ = DPC // 4      # quads per core
NS = 3             # bit-streams per packed byte

FP8 = ml_dtypes.float8_e4m3
MASKS = (0x4040, 0x2020, 0x0808)
SCALES = (0.5, 8.0, 64.0)   # coefficient prescale per stream (host side)

_nc_cache = {}

# weight chunk schedule: (quad, tile0, tile1) in consume order
CHUNKS = [
    (0, 0, 2), (0, 2, 4), (0, 4, 6),
    (1, 0, 2), (1, 2, 4), (1, 4, 6),
    (2, 0, 2), (2, 2, 4), (2, 4, 6),
    (3, 0, 2), (3, 2, 4), (3, 4, 6),
]
RING_OF = [0, 0, 0, 0, 1, 0, 1, 0, 1, 0, 1, 0]


def _emit(tc, res_ap, wT_ap, xT_ap, hdr_ap):
    """Emit the per-core program into TileContext tc."""
    import concourse.mybir as mybir

    nc = tc.nc
    fp8 = mybir.dt.float8e4
    u16 = mybir.dt.uint16
    f16 = mybir.dt.float16
    f32 = mybir.dt.float32
    u8 = mybir.dt.uint8
    XN = DPC * NT  # 96 coefficient columns per stream

    with (
        tc.tile_pool(name="w", bufs=1) as wp,
        tc.tile_pool(name="small", bufs=1) as sp,
        tc.tile_pool(name="act", bufs=1) as ap_pool,
        tc.tile_pool(name="ps", bufs=1, space="PSUM") as pp,
    ):
        # prescaled x coefficient streams, first on the SP ring:
        # xc[:, s*XN + d*NT + t] = SCALES[s] * x[d0+d, 3*(t*128+p) + s]
        xc = sp.tile([P, NS * XN], fp8)
        nc.sync.dma_start(out=xc[:], in_=xT_ap)
        # header on the ACT ring: kf = floor(bias) [4, NQ*O] ++ selneg [4,128]
        hdr = sp.tile([4, NQ * O + P], f16)
        nc.scalar.dma_start(out=hdr[:], in_=hdr_ap)

        ring = [nc.sync, nc.scalar]
        p_tiles = [wp.tile([P, NT * 4 * O], fp8, tag=f"p{q}", name=f"p_t{q}")
                   for q in range(NQ)]
        s_tiles = []
        for s in range(NS):
            row = []
            for q in range(NQ):
                t_ = wp.tile([P, NT * 4 * O], fp8, tag=f"s{s}q{q}",
                             name=f"s_t{s}_{q}")
                row.append(t_)
            s_tiles.append(row)
        for ci, (qi, t0, t1) in enumerate(CHUNKS):
            c0, c1 = t0 * 4 * O, t1 * 4 * O
            ring[RING_OF[ci]].dma_start(
                out=p_tiles[qi][:, c0:c1], in_=wT_ap[qi][:, c0:c1]
            )

        res_all = ap_pool.tile([P, NQ * O], u8)
        ps_all = pp.tile([P, 8 * 2 * O], f32)
        probe = sp.tile([1, 4], f32)

        # PE warm-up (HAM clock gate): ~3.5us of dummy matmuls into an
        # unused PSUM window before the first weight chunk lands.
        scratch = sp.tile([P, 2 * O], fp8)
        nc.vector.memset(scratch[:], 0.0)
        neg128 = sp.tile([P, 1], f32)
        nc.vector.memset(neg128[:], -128.0)
        for w in range(16):
            nc.tensor.matmul(
                ps_all[0:32, O : 2 * O],
                scratch[:, 0:32],
                scratch[:, 0:O],
                start=True,
                stop=True,
                tile_position=(0, 0),
                skip_group_check=True,
            )

        def derive(qi, t0, t1):
            c0, c1 = t0 * 4 * O, t1 * 4 * O
            for s in range(NS):
                nc.vector.tensor_scalar(
                    out=s_tiles[s][qi][:, c0:c1].bitcast(u16),
                    in0=p_tiles[qi][:, c0:c1].bitcast(u16),
                    scalar1=MASKS[s], scalar2=None,
                    op0=mybir.AluOpType.bitwise_and,
                )

        def mm_quad(q):
            win = slice(q * 2 * O, q * 2 * O + O)
            # -floor(bias) seeds the accumulation (integer => all partials
            # stay exact fp32 integers); keeping it FIRST removes it from
            # the kernel tail.
            nc.tensor.matmul(
                ps_all[:, win],
                hdr[0:4, NQ * O : NQ * O + P],
                hdr[0:4, q * O : (q + 1) * O],
                start=True,
                stop=False,
                skip_group_check=True,
            )
            # quads 0-2: stream-major (their derives finish mid-stream);
            # quad 3: tile-major so only the final small chunk's matmuls
            # trail the last DMA completion.
            if q == 3:
                order = [(t, s) for t in range(NT) for s in range(NS)]
            else:
                order = [(t, s) for s in range(NS) for t in range(NT)]
            for i, (t, s) in enumerate(order):
                for j in range(4):
                    d = q * 4 + j
                    nc.tensor.matmul(
                        ps_all[32 * j : 32 * (j + 1), win],
                        xc[:, s * XN + d * NT + t : s * XN + d * NT + t + 1]
                        .broadcast_to((P, 32)),
                        s_tiles[s][q][:, (t * 4 + j) * O : (t * 4 + j + 1) * O],
                        start=False,
                        stop=(i == len(order) - 1 and j == 3),
                        tile_position=(0, 32 * j),
                        skip_group_check=True,
                    )

        # Compares on the (otherwise idle) ACT engine so the DVE FIFO is a
        # pure derive chain (the derive chain is the measured critical path
        # from the first chunk semaphore to the last matmul). For integer
        # ps, act > bias <=> ps >= 1, and sigmoid(256*ps - 128) saturates
        # to exactly 1.0 / 0.0, so the uint8 cast is the exact predicate.
        # The probe triggers the one-time ~2.7us sigmoid table load early,
        # after the ACT-ring dma_starts.
        nc.scalar.activation(
            out=probe[:], in_=ps_all[0:1, O : O + 4],
            func=mybir.ActivationFunctionType.Sigmoid, scale=256.0,
            bias=neg128[0:1, :],
        )

        def compare(q):
            nc.scalar.activation(
                out=res_all[:, q * O : (q + 1) * O],
                in_=ps_all[:, q * 2 * O : q * 2 * O + O],
                func=mybir.ActivationFunctionType.Sigmoid,
                scale=256.0, bias=neg128[:],
            )

        derive(0, 0, 2)
        derive(0, 2, 4)
        derive(0, 4, 6)
        mm_quad(0)
        derive(1, 0, 2)
        derive(1, 2, 4)
        derive(1, 4, 6)
        compare(0)
        mm_quad(1)
        derive(2, 0, 2)
        derive(2, 2, 4)
        derive(2, 4, 6)
        compare(1)
        mm_quad(2)
        derive(3, 0, 2)
        derive(3, 2, 4)
        derive(3, 4, 6)
        compare(2)
        mm_quad(3)
        # bulk of the results flies out while quad 3 still computes; only
        # the last quad's 1KB rides the final DMA receipt
        nc.scalar.dma_start(out=res_ap[:, : 3 * O], in_=res_all[0:P:32, : 3 * O])
        compare(3)
        nc.scalar.dma_start(out=res_ap[:, 3 * O :], in_=res_all[0:P:32, 3 * O :])


def _build():
    """Build the per-core Bass program (same NEFF on all 8 cores)."""
    import concourse.bacc as bacc
    import concourse.mybir as mybir
    from concourse.tile import TileContext

    nc = bacc.Bacc("TRN2", debug=False, enable_asserts=False)

    fp8 = mybir.dt.float8e4
    f16 = mybir.dt.float16
    u8 = mybir.dt.uint8

    # wT[q, p, (t*4 + j)*O + o] = packed triple for direction d0+4q+j,
    # triple index m = t*128 + p, byte w[3m]*0x40 | w[3m+1]*0x20 | w[3m+2]*0x08
    wT = nc.dram_tensor("wT", [NQ, P, NT * 4 * O], fp8, kind="ExternalInput")
    # xT[p, s*96 + d*6 + t] = SCALES[s] * x[d0+d, 3*(t*128+p) + s]
    xT = nc.dram_tensor("xT", [P, NS * DPC * NT], fp8, kind="ExternalInput")
    # hdr[j, q*O + o] = floor(bias_noise[d0+4q+j, o]); hdr[j, NQ*O + m] =
    # -1.0 if m//32 == j else 0 (the bias-fold selector)
    hdr = nc.dram_tensor("hdr", [4, NQ * O + P], f16, kind="ExternalInput")
    # res[j, q*O + o] = out[d0+4q+j, o]
    res = nc.dram_tensor("res", [4, NQ * O], u8, kind="ExternalOutput")

    with TileContext(nc) as tc:
        _emit(tc, res.ap(), wT.ap(), xT.ap(), hdr.ap())
    nc.compile()
    return nc


def prepare_inputs(weight_noise, bias_noise, x):
    """Host-side dtype cast + bit packing + layout transform + sharding.

    All transforms are data-independent (fixed index shuffles, the exact
    0/1 bit pack into encoding fields, fixed scalar prescale of the 0/1
    x-bits, and floor() of the compare constant); the reduction/compare
    math runs on device.
    """
    w = np.asarray(weight_noise)                      # [D, O, K] 0/1 floats
    wtri = w.reshape(D, O, K // 3, 3).astype(np.uint8)
    enc = (wtri[..., 0] * 0x40 + wtri[..., 1] * 0x20 + wtri[..., 2] * 0x08
           ).astype(np.uint8).view(FP8)               # [D, O, 768]
    # [D, O, NT, P] -> [D, P, NT, O]
    pT = np.ascontiguousarray(enc.reshape(D, O, NT, P).transpose(0, 3, 2, 1))

    xb = np.asarray(x).astype(np.float32)             # [D, K] 0/1
    xs = []
    for s in range(NS):
        v = (xb[:, s::3] * SCALES[s]).astype(FP8).reshape(D, NT, P)
        xs.append(np.ascontiguousarray(v.transpose(2, 0, 1)))  # [P, D, NT]

    kf = np.floor(np.asarray(bias_noise).astype(np.float64)).astype(np.float16)
    selneg = np.zeros((4, P), dtype=np.float16)
    for j in range(4):
        selneg[j, 32 * j : 32 * (j + 1)] = -1.0

    in_maps = []
    for c in range(NCORES):
        sl = slice(c * DPC, (c + 1) * DPC)
        # [d, p, t, o] -> [q, j, p, t, o] -> [q, p, t, j, o]
        wc = (
            pT[sl]
            .reshape(NQ, 4, P, NT, O)
            .transpose(0, 2, 3, 1, 4)
            .reshape(NQ, P, NT * 4 * O)
        )
        xcs = np.concatenate(
            [xs[s][:, sl, :].reshape(P, DPC * NT) for s in range(NS)], axis=1
        )
        kc = (
            kf[sl]
            .reshape(NQ, 4, O)
            .transpose(1, 0, 2)
            .reshape(4, NQ * O)
        )
        hc = np.concatenate([kc, selneg], axis=1)
        in_maps.append(
            {
                "wT": np.ascontiguousarray(wc),
                "xT": np.ascontiguousarray(xcs),
                "hdr": np.ascontiguousarray(hc),
            }
        )
    return in_maps


def run(weight_noise, bias_noise, x, trace=False, **spmd_kwargs):
    """Run on the 8 NeuronCores; returns (bool [D, O] output, results)."""
    from concourse.bass_utils import run_bass_kernel_spmd

    in_maps = prepare_inputs(weight_noise, bias_noise, x)
    if "nc" in _nc_cache:
        nc = _nc_cache["nc"]
    else:
        nc = _nc_cache["nc"] = _build()
    r = run_bass_kernel_spmd(
        nc, in_maps, core_ids=list(range(NCORES)), trace=trace, **spmd_kwargs
    )
    out = np.concatenate(
        [
            r.results[c]["res"]
            .reshape(4, NQ, O)
            .transpose(1, 0, 2)
            .reshape(DPC, O)
            for c in range(NCORES)
        ],
        axis=0,
    )
    return out.astype(bool), r


def kernel(weight_noise, bias_noise, x):
    out, _ = run(weight_noise, bias_noise, x)
    return out
